# revision 11
# baseline (speedup 1.0000x reference)
"""Trainium2 Bass kernel for nn_CAM (channel attention module).

Reference (per batch b):
    f = x[b].reshape(N, C)                      # N = H*W = 4096, C = 512
    G = f^T f                                   # (C, C) channel gram
    A = softmax(G, axis=-1)
    out[b] = gamma * (f @ A) + x[b]

Algebraic folds used:
  * residual: x[b].reshape(N, C) == f, so out[b] = f @ (gamma * A + I);
    the residual add becomes part of the second matmul's stationary operand.
  * symmetry: G == G^T, so the gram phase only computes the upper-triangular
    128-blocks (row-block m covers columns >= 128*m, free dims 512/384/256/128)
    and the 6 lower blocks are reconstructed with cheap PE transposes.

Sharding: pure data-parallel over batch: 16 batches -> 8 cores x 2 batches.
Each core runs the identical program on its own 2-batch shard; gamma and a
512x512 identity constant are replicated.

Per-core dataflow (per batch):
  1. SWDGE DMA loads x and casts fp32 -> bf16 in flight into `fb`.
  2. Triangular gram into 4 PSUM tiles (contraction over 32 spatial chunks).
  3. PSUM -> SBUF copies, 6 fp32 PE transposes to mirror the lower blocks.
  4. Softmax over rows of G: reduce_max (DVE), Exp with -max bias (ACT,
     row sums via accum_out), reciprocal, then B = (gamma/sum)*E + I in one
     scalar_tensor_tensor (bf16).
  5. PE-transpose all 128x128 blocks of fb into `ft` (= f^T, bf16) -- placed
     after the gram so these matmuls hide the softmax latency.
  6. Matmul 2: out_tile(128n, 512) = sum_m ft[m-block]^T @ B[m] (PSUM fp32),
     copy to SBUF (DVE), DMA out.
"""

import sys

if "/opt/trn_rl_repo" not in sys.path:
    sys.path.insert(0, "/opt/trn_rl_repo")

import numpy as np
import ml_dtypes

import concourse.bacc as bacc
import concourse.mybir as mybir
import concourse.tile as tile
from concourse.alu_op_type import AluOpType
from concourse.bass_utils import run_bass_kernel_spmd

F32 = mybir.dt.float32
BF16 = mybir.dt.bfloat16
AF = mybir.ActivationFunctionType

N_CORES = 8
B_FULL, H, W, C = 16, 64, 64, 512
N = H * W                      # 4096 spatial positions per batch
B_LOC = B_FULL // N_CORES      # 2 batches per core


def build_nc(b_loc=B_LOC, n=N, c=C, num_devices=N_CORES, reps=None,
             dma_cast=True, tri_gram=True, ft_via="pe", fp8_gram=False):
    """Build + compile the per-core Bass program.

    reps: if set, wrap the whole body in a hardware For_i loop (timing builds).
    """
    nk = n // 128   # 128-row spatial chunks
    nm = c // 128   # 128-row channel blocks

    nc = bacc.Bacc(
        "TRN2",
        target_bir_lowering=False,
        debug=False,
        num_devices=num_devices,
    )

    x_d = nc.dram_tensor("x", [b_loc * n, c], F32, kind="ExternalInput")
    gam_d = nc.dram_tensor("gamma", [1, 1], F32, kind="ExternalInput")
    id_d = nc.dram_tensor("ident", [c, c], BF16, kind="ExternalInput")
    y_d = nc.dram_tensor("y", [b_loc * n, c], F32, kind="ExternalOutput")

    with tile.TileContext(nc) as tc:
        with (
            tc.tile_pool(name="xin", bufs=6) as p_xin,
            tc.tile_pool(name="fb", bufs=2) as p_fb,
            tc.tile_pool(name="ft", bufs=2) as p_ft,
            tc.tile_pool(name="gsb", bufs=2 * nm) as p_g,
            tc.tile_pool(name="esb", bufs=2 * nm) as p_e,
            tc.tile_pool(name="bsb", bufs=2 * nm) as p_b,
            tc.tile_pool(name="stat", bufs=8 * nm) as p_stat,
            tc.tile_pool(name="outp", bufs=6) as p_out,
            tc.tile_pool(name="const", bufs=1) as p_const,
            tc.tile_pool(name="psg", bufs=1, space="PSUM") as p_psg,
            tc.tile_pool(name="pst", bufs=2, space="PSUM") as p_pst,
            tc.tile_pool(name="pso", bufs=2, space="PSUM") as p_pso,
        ):
            def body(_iv=None):
                # --- constants ---
                ident_rows = []
                for m in range(nm):
                    t = p_const.tile([128, c], BF16, tag=f"ident{m}",
                                     name=f"ident{m}")
                    nc.sync.dma_start(out=t[:, :],
                                      in_=id_d[m * 128:(m + 1) * 128, :])
                    ident_rows.append(t)
                ident128 = ident_rows[0][:, 0:128]
                idf32 = p_const.tile([128, 128], F32, tag="idf32", name="idf32")
                nc.vector.tensor_copy(idf32[:, :], ident128)

                gam1 = p_const.tile([1, 1], F32, tag="gam1", name="gam1")
                nc.sync.dma_start(out=gam1[:, :], in_=gam_d[:, :])
                gamb = p_const.tile([128, 1], F32, tag="gamb", name="gamb")
                nc.gpsimd.partition_broadcast(gamb[:, :], gam1[:, :])

                for b in range(b_loc):
                    # --- load (+cast) ---
                    fb = p_fb.tile([128, nk * c], BF16, tag="fb", name=f"fb{b}")
                    if dma_cast:
                        # ramp the first batch's groups so the first gram
                        # matmul isn't stalled behind a 2MB descriptor
                        if b == 0:
                            sizes = [1, 1, 2] + [4] * ((nk - 4) // 4)
                        else:
                            sizes = [4] * (nk // 4)
                        k0 = 0
                        for grp in sizes:
                            src = x_d[b * n + k0 * 128:
                                      b * n + (k0 + grp) * 128, :]
                            dst = fb[:, k0 * c:(k0 + grp) * c]
                            nc.gpsimd.dma_start(
                                out=dst.rearrange("p (j c1) -> p j c1", j=grp),
                                in_=src.rearrange("(j p) c1 -> p j c1", p=128),
                            )
                            k0 += grp
                        assert k0 == nk
                    else:
                        for k in range(nk):
                            xt = p_xin.tile([128, c], F32, tag="xin",
                                            name=f"x{b}_{k}")
                            nc.sync.dma_start(
                                out=xt[:, :],
                                in_=x_d[b * n + k * 128: b * n + (k + 1) * 128, :],
                            )
                            nc.vector.tensor_copy(fb[:, k * c:(k + 1) * c],
                                                  xt[:, :])

                    # --- gram (triangular) ---
                    ps_g = [p_psg.tile([128, c], F32, tag=f"psg{m}",
                                       name=f"psg{m}_{b}") for m in range(nm)]
                    if fp8_gram:
                        # fp8e4 copy of f; DoubleRow packs 2 spatial chunks
                        # per matmul (K=256) at 2x ALU rate. Gram precision
                        # is insensitive: softmax saturates on the diagonal.
                        f8 = p_fb.tile([128, nk * c], mybir.dt.float8e4,
                                       tag="f8", name=f"f8{b}", bufs=1)
                        for k in range(nk):
                            nc.vector.tensor_copy(f8[:, k * c:(k + 1) * c],
                                                  fb[:, k * c:(k + 1) * c])
                        for kp in range(nk // 2):
                            sl = (f8[:, 2 * kp * c:(2 * kp + 2) * c]
                                  .rearrange("p (o c1) -> p o c1", o=2))
                            for m in range(nm):
                                lo = m * 128 if tri_gram else 0
                                nc.tensor.matmul(
                                    ps_g[m][:, lo:c],
                                    sl[:, :, m * 128:(m + 1) * 128],
                                    sl[:, :, lo:c],
                                    start=(kp == 0),
                                    stop=(kp == nk // 2 - 1),
                                    perf_mode=mybir.MatmulPerfMode.DoubleRow,
                                )
                    else:
                        for k in range(nk):
                            fbk = fb[:, k * c:(k + 1) * c]
                            for m in range(nm):
                                lo = m * 128 if tri_gram else 0
                                nc.tensor.matmul(
                                    ps_g[m][:, lo:c],
                                    fbk[:, m * 128:(m + 1) * 128],
                                    fbk[:, lo:c],
                                    start=(k == 0),
                                    stop=(k == nk - 1),
                                )

                    # --- G psum -> sbuf, mirror lower blocks ---
                    g_sb = []
                    for m in range(nm):
                        t = p_g.tile([128, c], F32, tag="gsb", name=f"g{b}_{m}")
                        lo = m * 128 if tri_gram else 0
                        nc.vector.tensor_copy(t[:, lo:c], ps_g[m][:, lo:c])
                        g_sb.append(t)

                    ft = p_ft.tile([128, nm, n], BF16, tag="ft", name=f"ft{b}")

                    def ftr(k):
                        fbk = fb[:, k * c:(k + 1) * c]
                        if ft_via == "dma":
                            # xbar transpose: out[p, m, j] = fbk[j, m*128+p]
                            nc.sync.dma_start_transpose(
                                ft[:, :, k * 128:(k + 1) * 128], fbk,
                            )
                            return
                        ps_t = p_pst.tile([128, c], BF16, tag="pst",
                                          name=f"pst{b}_{k}")
                        for m in range(nm):
                            nc.tensor.transpose(
                                ps_t[:, m * 128:(m + 1) * 128],
                                fbk[:, m * 128:(m + 1) * 128],
                                ident128,
                            )
                        # ft[p, m, k*128 + j] = f[k*128 + j, m*128 + p]
                        nc.scalar.copy(
                            ft[:, :, k * 128:(k + 1) * 128],
                            ps_t[:, :].rearrange("p (m j) -> p m j", m=nm),
                        )

                    # a few f-transposes first so PE isn't stalled on the
                    # DVE g-copies, then the 6 G mirrors, then the rest.
                    pre = min(8, nk)
                    for k in range(pre):
                        ftr(k)
                    if tri_gram:
                        for m in range(1, nm):
                            for d in range(m):
                                tp = p_pso.tile([128, 128], F32, tag="pso",
                                                name=f"gt{b}_{m}_{d}")
                                nc.tensor.transpose(
                                    tp[:, :],
                                    g_sb[d][:, m * 128:(m + 1) * 128],
                                    idf32[:, :],
                                )
                                nc.vector.tensor_copy(
                                    g_sb[m][:, d * 128:(d + 1) * 128], tp[:, :])

                    # --- softmax + B = (gamma/sum)*E + I ---
                    b_rows = []
                    for m in range(nm):
                        nmax = p_stat.tile([128, 1], F32, tag="nmax",
                                           name=f"nmax{b}_{m}")
                        nc.vector.reduce_max(
                            nmax[:, :], g_sb[m][:, :], axis=mybir.AxisListType.X,
                            negate=True,
                        )
                        e_sb = p_e.tile([128, c], BF16, tag="esb",
                                        name=f"e{b}_{m}")
                        esum = p_stat.tile([128, 1], F32, tag="esum",
                                           name=f"esum{b}_{m}")
                        nc.scalar.activation(
                            e_sb[:, :], g_sb[m][:, :], AF.Exp,
                            bias=nmax[:, :], scale=1.0, accum_out=esum[:, :],
                        )
                        rec = p_stat.tile([128, 1], F32, tag="rec",
                                          name=f"rec{b}_{m}")
                        nc.vector.reciprocal(rec[:, :], esum[:, :])
                        sc = p_stat.tile([128, 1], F32, tag="sc",
                                         name=f"sc{b}_{m}")
                        nc.vector.tensor_tensor(
                            sc[:, :], rec[:, :], gamb[:, :], op=AluOpType.mult,
                        )
                        b_sb = p_b.tile([128, c], BF16, tag="bsb",
                                        name=f"bmat{b}_{m}")
                        nc.vector.scalar_tensor_tensor(
                            b_sb[:, :], e_sb[:, :], sc[:, :],
                            ident_rows[m][:, :],
                            op0=AluOpType.mult, op1=AluOpType.add,
                        )
                        b_rows.append(b_sb)

                    for k in range(pre, nk):
                        ftr(k)

                    # --- out = f @ B ---
                    for t in range(nk):
                        ps_o = p_pso.tile([128, c], F32, tag="pso",
                                          name=f"pso{b}_{t}")
                        for m in range(nm):
                            nc.tensor.matmul(
                                ps_o[:, :],
                                ft[:, m, t * 128:(t + 1) * 128],
                                b_rows[m][:, :],
                                start=(m == 0),
                                stop=(m == nm - 1),
                            )
                        o_sb = p_out.tile([128, c], F32, tag="outp",
                                          name=f"o{b}_{t}")
                        nc.vector.tensor_copy(o_sb[:, :], ps_o[:, :])
                        nc.sync.dma_start(
                            out=y_d[b * n + t * 128: b * n + (t + 1) * 128, :],
                            in_=o_sb[:, :],
                        )

            if reps is None:
                body()
            else:
                with tc.For_i(0, reps, 1,
                              hint_engines=(mybir.EngineType.PE,
                                            mybir.EngineType.DVE,
                                            mybir.EngineType.Activation)) as iv:
                    body(iv)

    nc.compile()
    return nc


_NC_CACHE = {}


def _get_nc():
    if "full" not in _NC_CACHE:
        _NC_CACHE["full"] = build_nc()
    return _NC_CACHE["full"]


def make_in_maps(inputs_np, gamma_np):
    """Shard full inputs into per-core in_maps."""
    x = np.ascontiguousarray(
        np.asarray(inputs_np, dtype=np.float32).reshape(B_FULL, N, C)
    )
    gam = np.asarray(gamma_np, dtype=np.float32).reshape(1, 1)
    ident = np.eye(C, dtype=np.float32).astype(ml_dtypes.bfloat16)
    in_maps = []
    for core in range(N_CORES):
        xs = x[core * B_LOC:(core + 1) * B_LOC].reshape(B_LOC * N, C)
        in_maps.append({
            "x": np.ascontiguousarray(xs),
            "gamma": gam,
            "ident": ident,
        })
    return in_maps


def kernel(inputs, gamma):
    nc = _get_nc()
    in_maps = make_in_maps(inputs, gamma)
    res = run_bass_kernel_spmd(nc, in_maps, core_ids=list(range(N_CORES)))
    outs = [res.results[c]["y"].reshape(B_LOC, N, C) for c in range(N_CORES)]
    y = np.concatenate(outs, axis=0).reshape(B_FULL, H, W, C)
    return y.astype(np.float32)
